# revision 74
# baseline (speedup 1.0000x reference)
"""Trainium2 Bass kernel for nn_Cross_classifier (dense_cnn).

Pure data-parallel: batch 128 sharded across 8 NeuronCores (16 samples/core).
All parameters replicated. Self-contained: shapes hardcoded.

Math (mirrors the reference):
  - f_z: Linear(1536->384) + LayerNorm + GELU on z = concat(z_r, z_i).
  - down_r/down_i: 3x3 SAME conv (768->384) + eval-BN + GELU, center-crop
    16x16 -> 8x8.  Only the central 8x8 outputs are consumed, so the conv is
    computed only there from the central 10x10 input patch.  BN scale folds
    into the conv weights; conv bias + BN shift fold into one per-channel
    bias applied inside the GELU activation.
  - xcorr: VALID correlation of an 8x8 kernel over an 8x8 map = per-sample
    dot over (384 ch x 64 pos); then sigmoid(dot / c).

Implementation:
  - Every matmul runs fp8e4m3 x fp8e4m3 in MatmulPerfMode.DoubleRow (two
    128-deep K-subtiles per pass), accumulating in fp32 PSUM.  The final
    sigmoid sits at ~sigmoid(10), so fp8 rounding is far inside tolerance.
    Weights are scaled x32 on the host to center them in fp8e4m3's normal
    range; the scale is removed exactly (LayerNorm is scale-invariant for
    f_z; the conv GELU applies scale=1/32).
  - All layout work (transposes to contraction-major, weight folding, fp8
    casts) happens on the host, so the device program is load -> matmul ->
    activation -> reduce, plus one small bf16 xbar transpose of z_f.
  - Conv moving APs must fit TENSOR3D (3 free dims): per-sample tap windows
    [K, ksub(2), row(8), col(8)] over a 10x10 patch, 64-wide output slices
    of a shared PSUM tile per sample.
  - Engine program order keeps the serial DMA stream and the PE stream in
    lockstep: warmup matmuls burn the PE p-state ramp while the first
    chunk loads, then conv-r k0/k1, f_z, conv-r k2, conv-i.  The LayerNorm
    mean comes free from a 385th fzw column of row-sums; variance uses one
    fused square-accumulate per token tile; the normalize folds into the
    GELU's per-partition bias/scale.  The final sigmoids run as
    0.5*(1+tanh(x/2)) so they share the GELU activation table (the Sqrt
    table is preloaded at t=0), keeping table loads off the critical path.
"""

import numpy as np
import ml_dtypes

N_CORES = 8
B = 128
BPC = B // N_CORES      # samples per core: 16
T1 = 64                 # template tokens (8x8)
E = 768
E2 = 384
TWOE = 2 * E            # 1536
KCPZ = TWOE // 256      # 6 DoubleRow K-chunks for f_z
KCP = E // 256          # 3 DoubleRow K-chunks for conv
MC = E2 // 128          # 3 output-channel chunks
TOK = BPC * T1          # 1024 z tokens per core
NZT = TOK // 128        # 8 token tiles
GRP = BPC // 8          # sample groups of 8 (one PSUM tile each)
NP = 100                # patch elems/sample: 10 rows x 10 cols
SC = 32.0               # fp8 weight scale
EPS = 1e-5

BF16 = ml_dtypes.bfloat16
FP8 = ml_dtypes.float8_e4m3

_PROG_CACHE: dict = {}


def _build_program(flags):
    """flags = (has_fzb, has_lng, has_lnb): whether the f_z linear bias /
    LayerNorm gain / LayerNorm bias are non-trivial (structurally zero/one
    in this model; general path kept for robustness)."""
    from contextlib import ExitStack
    import concourse.bass as bass
    import concourse.mybir as mybir
    import concourse.tile as tile
    from concourse import bacc

    has_fzb, has_lng, has_lnb = flags
    dt = mybir.dt
    f32, bf16, fp8 = dt.float32, dt.bfloat16, dt.float8e4
    AX = mybir.AxisListType
    OP = mybir.AluOpType
    AF = mybir.ActivationFunctionType
    DR = mybir.MatmulPerfMode.DoubleRow

    nc = bacc.Bacc("TRN2", target_bir_lowering=False, debug=False,
                   num_devices=N_CORES)

    # ---- DRAM I/O (layouts: every DMA <=3 affine dims, >=512B runs) ----
    z_d = nc.dram_tensor("z", [2, KCPZ, 128, 2, TOK // 2], fp8,
                         kind="ExternalInput")
    fzw_d = nc.dram_tensor("fzw", [KCPZ, 128, 2, E2 + 1], fp8,
                           kind="ExternalInput")
    xr_d = nc.dram_tensor("xr", [KCP, 2, 128, BPC * NP], fp8,
                          kind="ExternalInput")
    xi_d = nc.dram_tensor("xi", [KCP, 2, 128, BPC * NP], fp8,
                          kind="ExternalInput")
    wr_d = nc.dram_tensor("wr", [KCP, MC, 128, 2 * 9 * 128], fp8,
                          kind="ExternalInput")
    wi_d = nc.dram_tensor("wi", [KCP, MC, 128, 2 * 9 * 128], fp8,
                          kind="ExternalInput")
    bshr_d = nc.dram_tensor("bshr", [MC, 128], f32, kind="ExternalInput")
    bshi_d = nc.dram_tensor("bshi", [MC, 128], f32, kind="ExternalInput")
    ones_d = nc.dram_tensor("ones", [128, 1], bf16, kind="ExternalInput")
    c_d = nc.dram_tensor("c", [1, 1], f32, kind="ExternalInput")
    fzb_d = nc.dram_tensor("fzb", [1, E2 + 1], f32, kind="ExternalInput")
    lng_d = nc.dram_tensor("lng", [1, E2], f32, kind="ExternalInput")
    lnb_d = nc.dram_tensor("lnb", [1, E2], f32, kind="ExternalInput")
    s1_d = nc.dram_tensor("s1", [1, BPC], f32, kind="ExternalOutput")
    s2_d = nc.dram_tensor("s2", [1, BPC], f32, kind="ExternalOutput")

    def bcast_ap(handle):
        ap = handle.ap()
        return bass.AP(tensor=ap.tensor, offset=ap.offset,
                       ap=[[0, 128]] + [list(d) for d in ap.ap[1:]])

    with tile.TileContext(nc, pool_alloc_mode="queue") as tc, ExitStack() as ctx:
        const = ctx.enter_context(tc.tile_pool(name="const", bufs=1))

        # consts ride the DVE ring so the SP ring starts the big loads at
        # t=0; each is tiny and slots between big transfers.
        onesb = const.tile([128, 1], bf16)
        nc.scalar.dma_start(out=onesb, in_=ones_d.ap())
        ctile = const.tile([1, 1], f32)
        nc.scalar.dma_start(out=ctile, in_=c_d.ap())
        invc2 = const.tile([1, 1], f32)
        nc.vector.tensor_scalar_mul(out=invc2, in0=ctile, scalar1=2.0)
        nc.vector.reciprocal(invc2, invc2)
        bshr = const.tile([128, MC], f32)
        nc.scalar.dma_start(out=bshr, in_=bshr_d.ap().rearrange("m p -> p m"))
        bshi = const.tile([128, MC], f32)
        nc.scalar.dma_start(out=bshi, in_=bshi_d.ap().rearrange("m p -> p m"))
        epst = const.tile([128, 1], f32)
        nc.vector.memset(epst, EPS * SC * SC)  # eps for x32-scaled variance
        sqwarm = const.tile([128, 1], f32)
        nc.scalar.activation(out=sqwarm, in_=epst, func=AF.Sqrt)

        if has_fzb:
            fzb_bc = const.tile([128, E2 + 1], f32)
            nc.scalar.dma_start(out=fzb_bc, in_=bcast_ap(fzb_d))
        if has_lng:
            lng_bc = const.tile([128, E2], f32)
            nc.scalar.dma_start(out=lng_bc, in_=bcast_ap(lng_d))
        if has_lnb:
            lnb_bc = const.tile([128, E2], f32)
            nc.scalar.dma_start(out=lnb_bc, in_=bcast_ap(lnb_d))

        # ---- persistent SBUF tiles ----
        data = ctx.enter_context(tc.tile_pool(name="data", bufs=1))
        zt = data.tile([128, 2, KCPZ, 2, TOK // 2], fp8)  # z.T, K-major
        fzw = data.tile([128, KCPZ, 2, E2 + 1], fp8)
        XTr = data.tile([128, KCP, 2, BPC * NP], fp8)
        XTi = data.tile([128, KCP, 2, BPC * NP], fp8)
        Wr = data.tile([128, KCP, MC, 2, 9, 128], fp8)
        Wi = data.tile([128, KCP, MC, 2, 9, 128], fp8)
        zg2all = data.tile([128, NZT, E2], bf16)        # gelu(LN(f_z))
        ZGT = data.tile([128, NZT, MC, 128], bf16)      # ch-major z_f
        zlin = data.tile([128, NZT, E2 + 1], f32)       # f_z lin out + mu

        fin_pool = ctx.enter_context(tc.tile_pool(name="fin", bufs=1))
        cps = ctx.enter_context(tc.tile_pool(name="cps", bufs=6, space="PSUM"))
        xgp = ctx.enter_context(tc.tile_pool(name="xg", bufs=12))
        xcp = ctx.enter_context(tc.tile_pool(name="xc", bufs=4))

        # ---- input DMA stream (SP ring, consumption order: conv-r k0,
        # z-phase tensors, conv-r k1/k2, conv-i) ----
        for k in range(2):
            nc.sync.dma_start(
                out=XTr[:, k, :, :],
                in_=xr_d.ap()[k].rearrange("t p q -> p t q"))
            for mc in range(MC):
                nc.sync.dma_start(out=Wr[:, k, mc, :, :, :],
                                  in_=wr_d.ap()[k, mc])
        nc.sync.dma_start(out=fzw,
                          in_=fzw_d.ap().rearrange("k p t e -> p k t e"))
        for h in range(2):
            nc.sync.dma_start(
                out=zt[:, h], in_=z_d.ap()[h].rearrange("k p t n -> p k t n"))
        nc.sync.dma_start(
            out=XTr[:, 2, :, :],
            in_=xr_d.ap()[2].rearrange("t p q -> p t q"))
        for mc in range(MC):
            nc.sync.dma_start(out=Wr[:, 2, mc, :, :, :],
                              in_=wr_d.ap()[2, mc])
        iloads = []
        for k in range(KCP):
            li = nc.sync.dma_start(
                out=XTi[:, k, :, :],
                in_=xi_d.ap()[k].rearrange("t p q -> p t q"))
            if k == 2:
                iloads.append(li)
            for mc in range(MC):
                li = nc.sync.dma_start(out=Wi[:, k, mc, :, :, :],
                                       in_=wi_d.ap()[k, mc])
                if k == 2:
                    iloads.append(li)

        # ---- conv matmul phase (per branch): per-sample DoubleRow taps ----
        def conv_mm(tag, XT, W, pcs, kcps):
            for kcp in kcps:
                for g in range(GRP):
                    for mc in range(MC):
                        if kcp == 0 and (g, mc) not in pcs:
                            pcs[(g, mc)] = cps.tile([128, 512], f32,
                                                    name="pc", tag="pc")
                        pc = pcs[(g, mc)]
                        for si in range(8):
                            s = g * 8 + si
                            for tap in range(9):
                                dy, dx = tap // 3, tap % 3
                                off = (XT.offset + kcp * (2 * BPC * NP)
                                       + s * NP + dy * 10 + dx)
                                rhs = bass.AP(
                                    tensor=XT.tensor, offset=off,
                                    ap=[list(XT.ap[0]), [BPC * NP, 2],
                                        [10, 8], [1, 8]])
                                nc.tensor.matmul(
                                    pc[:, si * T1:(si + 1) * T1],
                                    lhsT=W[:, kcp, mc, :, tap, :],
                                    rhs=rhs,
                                    start=(kcp == 0 and tap == 0),
                                    stop=(kcp == KCP - 1 and tap == 8),
                                    perf_mode=DR,
                                    skip_group_check=True)

        # gelu(conv/32 + shift) frees the PSUM banks early
        def conv_gelu(tag, pcs, bsh, gate=None, skip=()):
            xgs = {}
            for g in range(GRP):
                for mc in range(MC):
                    if (g, mc) in skip:
                        continue
                    xg = xgp.tile([128, 512], bf16, name="xg", tag="xg")
                    inst = nc.scalar.activation(out=xg, in_=pcs[(g, mc)],
                                                func=AF.Gelu,
                                                bias=bsh[:, mc:mc + 1],
                                                scale=1.0 / SC)
                    if gate is not None:
                        tile.add_dep_helper(inst.ins, gate, sync=True,
                                            reason="sqrt before gelu table")
                    xgs[(g, mc)] = xg
                    xgs["last"] = inst
            return xgs

        # xcorr: per-sample dot with z_f (mul on DVE; reduce on Pool for
        # branch r where it hides under conv-i, on DVE for branch i), then
        # the cross-partition sum accumulates straight into a [1, 16] PSUM
        # via tiny ones-matmuls (one per (g, mc), mc-accumulated).
        def conv_xcorr(tag, xgs, dot, red_eng, skip=()):
            for g in range(GRP):
                for mc in range(MC):
                    if (g, mc) in skip:
                        continue
                    xg = xgs[(g, mc)]
                    prod = xcp.tile([128, 4, 128], bf16, name="prod",
                                    tag="prod", bufs=6)
                    nc.vector.tensor_mul(
                        prod, xg.rearrange("p (a b) -> p a b", a=4),
                        ZGT[:, 4 * g:4 * g + 4, mc, :])
                    red = xcp.tile([128, 8], bf16, name="red", tag="red",
                                   bufs=12)
                    with nc.allow_low_precision(
                            reason="64-term bf16 sums; sigmoid(~10) output "
                                   "tolerates ~1e-2 logit error"):
                        red_eng.tensor_reduce(
                            out=red,
                            in_=prod.rearrange("p a b -> p (a b)").rearrange(
                                "p (s q) -> p s q", q=T1),
                            axis=AX.X, op=OP.add)
                    nc.tensor.matmul(dot[:, g * 8:(g + 1) * 8],
                                     lhsT=onesb, rhs=red,
                                     start=(mc == 0), stop=(mc == MC - 1),
                                     skip_group_check=True)

        # PE order: warmup (burn the p-state ramp while the first chunk
        # loads), conv-r k0, f_z (its LN/act/transpose chain then hides
        # under the remaining conv matmuls), conv-r k1/k2, conv-i.
        junk = data.tile([128, 2, 640], fp8)
        nc.vector.memset(junk, 0.0)
        pcs_r: dict = {}
        for mc in range(MC):
            for g in range(GRP):
                pcs_r[(g, mc)] = cps.tile([128, 512], f32, name="pc",
                                          tag="pc")
        for w in range(22):
            nc.tensor.matmul(pcs_r[(0, 0)], lhsT=junk[:, :, 0:128],
                             rhs=junk[:, :, 0:512], start=True, stop=True,
                             perf_mode=DR, skip_group_check=True)
        conv_mm("r", XTr, Wr, pcs_r, [0, 1])

        # ---------------- f_z: Linear + LayerNorm pass 1 ----------------
        zsp = ctx.enter_context(tc.tile_pool(name="zstat", bufs=4))
        zgp = ctx.enter_context(tc.tile_pool(name="zg", bufs=2))
        fzps = ctx.enter_context(
            tc.tile_pool(name="fzps", bufs=2, space="PSUM"))
        # LN stats: mu comes free as fzw's appended 385th column
        # (row-sums / 384); Sum x^2 via one fused square-accumulate per
        # token tile on DVE.
        rst = zsp.tile([128, NZT], f32, tag="rst", bufs=1)
        ssq = zsp.tile([128, NZT], f32, tag="ssq", bufs=1)
        for tt in range(NZT):
            ps = fzps.tile([128, E2 + 1], f32, name="ps", tag="ps")
            for kcp in range(KCPZ):
                nc.tensor.matmul(
                    ps,
                    lhsT=zt[:, tt // 4, kcp, :,
                            (tt % 4) * 128:(tt % 4 + 1) * 128],
                    rhs=fzw[:, kcp, :, :],
                    start=(kcp == 0), stop=(kcp == KCPZ - 1),
                    perf_mode=DR)
            # copy out (on Act: table-free Copy) so the PSUM bank frees
            # fast (2-bank pool); DVE does only the sum-of-squares
            nc.scalar.copy(out=zlin[:, tt, :], in_=ps)
            if has_fzb:
                nc.vector.tensor_add(zlin[:, tt, :], zlin[:, tt, :],
                                     fzb_bc)
            sq_src = zlin[:, tt, :E2]
            sq = zsp.tile([128, E2], bf16, tag="sqs", bufs=2)
            with nc.allow_low_precision(reason="x^2 scratch, accum is f32"):
                nc.vector.scalar_tensor_tensor(
                    out=sq, in0=sq_src, scalar=1.0,
                    in1=sq_src, op0=OP.mult, op1=OP.mult,
                    accum_out=ssq[:, tt:tt + 1])
        mus = zlin[:, :, E2]                       # [128, NZT], stride E2+1
        musq = zsp.tile([128, NZT], f32, tag="musq", bufs=1)
        nc.vector.tensor_tensor(out=musq, in0=mus, in1=mus, op=OP.mult)
        var8 = zsp.tile([128, NZT], f32, tag="var8", bufs=1)
        nc.vector.scalar_tensor_tensor(out=var8, in0=ssq,
                                       scalar=1.0 / E2, in1=musq,
                                       op0=OP.mult, op1=OP.subtract)
        sq_inst = nc.scalar.activation(out=rst, in_=var8,
                                       func=AF.Sqrt, bias=epst, scale=1.0)
        nc.vector.reciprocal(rst, rst)
        # nmr = -mu * rstd; LN then folds into gelu(rstd*x + nmr)
        # (tokens sit on partitions, so mu/rstd are per-partition)
        nmr = zsp.tile([128, NZT], f32, tag="nmr", bufs=1)
        nc.vector.tensor_tensor(out=nmr, in0=mus, in1=rst, op=OP.mult)
        nc.vector.tensor_scalar_mul(out=nmr, in0=nmr, scalar1=-1.0)

        conv_mm("r", XTr, Wr, pcs_r, [2])
        xgs_r = conv_gelu("r", pcs_r, bshr, gate=sq_inst.ins)

        # ---- f_z pass 2: GELU with folded LN affine ----
        zlast = [sq_inst.ins]
        if has_lng or has_lnb:
            zgtmp = zgp.tile([128, NZT, E2], bf16, tag="zgtmp", bufs=1)
            for tt in range(NZT):
                nc.vector.tensor_scalar(out=zgtmp[:, tt, :],
                                        in0=zlin[:, tt, :E2],
                                        scalar1=zlin[:, tt, E2:E2 + 1],
                                        scalar2=rst[:, tt:tt + 1],
                                        op0=OP.subtract, op1=OP.mult)
                if has_lng:
                    nc.vector.tensor_mul(zgtmp[:, tt, :],
                                         zgtmp[:, tt, :], lng_bc)
                if has_lnb:
                    nc.vector.tensor_add(zgtmp[:, tt, :],
                                         zgtmp[:, tt, :], lnb_bc)
            nc.scalar.activation(
                out=zg2all.rearrange("p a b -> p (a b)"),
                in_=zgtmp.rearrange("p a b -> p (a b)"), func=AF.Gelu)
        else:
            for tt in range(NZT):
                zi = nc.scalar.activation(out=zg2all[:, tt, :],
                                          in_=zlin[:, tt, :E2],
                                          func=AF.Gelu,
                                          bias=nmr[:, tt:tt + 1],
                                          scale=rst[:, tt:tt + 1])
                zlast[0] = zi.ins
                if tt == 3 or tt == NZT - 1:
                    # half-transposes (Act ring): sample-group g0's z_f is
                    # ready for xcorr early.  The conv-i chunk-(h+1) loads
                    # wait for the transpose so it gets a deterministic
                    # DMA-mutex slot instead of starving behind the load
                    # stream (the loads have ample slack).
                    h = tt // 4
                    ti = nc.scalar.dma_start_transpose(
                        ZGT[:, 4 * h:4 * h + 4, :, :],
                        zg2all[:, 4 * h:4 * h + 4, :])

        if has_lng or has_lnb:
            nc.scalar.dma_start_transpose(ZGT[:, :, :, :], zg2all)

        pcs_i: dict = {}
        conv_mm("i", XTi, Wi, pcs_i, [0, 1, 2])
        xgs_i = conv_gelu("i", pcs_i, bshi)

        dot_r = fzps.tile([1, BPC], f32, name="dotr", tag="ps")
        conv_xcorr("r", xgs_r, dot_r, nc.vector)
        sg_r = fin_pool.tile([1, BPC], f32, tag="sgr")
        nc.scalar.activation(out=sg_r, in_=dot_r, func=AF.Tanh,
                             scale=invc2[0:1, 0:1])
        nc.vector.tensor_scalar(out=sg_r, in0=sg_r, scalar1=0.5, scalar2=0.5,
                                op0=OP.mult, op1=OP.add)
        nc.sync.dma_start(out=s1_d.ap(), in_=sg_r)

        dot_i = fzps.tile([1, BPC], f32, name="doti", tag="ps")
        conv_xcorr("i", xgs_i, dot_i, nc.vector)
        sg_i = fin_pool.tile([1, BPC], f32, tag="sgi")
        nc.scalar.activation(out=sg_i, in_=dot_i, func=AF.Tanh,
                             scale=invc2[0:1, 0:1])
        nc.vector.tensor_scalar(out=sg_i, in0=sg_i, scalar1=0.5, scalar2=0.5,
                                op0=OP.mult, op1=OP.add)
        nc.sync.dma_start(out=s2_d.ap(), in_=sg_i)

    nc.finalize()
    return nc


def get_program(flags=(False, False, False)):
    if flags not in _PROG_CACHE:
        _PROG_CACHE[flags] = _build_program(flags)
    return _PROG_CACHE[flags]


def prep_inputs(z_r, z_i, x_r, x_i, fz_w, fz_b, ln_g, ln_b,
                wr, br, bnr_g, bnr_b, bnr_m, bnr_v,
                wi, bi, bni_g, bni_b, bni_m, bni_v, c):
    """Host-side sharding + packing. Returns (flags, in_maps)."""
    z_r = np.asarray(z_r, np.float32)
    z_i = np.asarray(z_i, np.float32)
    x_r = np.asarray(x_r, np.float32)
    x_i = np.asarray(x_i, np.float32)

    # template: z = concat(z_r, z_i) -> [B, 64, 1536]
    z = np.concatenate([z_r, z_i], axis=2)

    # search: central 10x10 patch, K-major fp8:
    # [kcp, ksub, p, core, samp, 100]
    def patch_pack(x):
        xg = x.transpose(0, 2, 1).reshape(B, E, 16, 16)
        patch = np.ascontiguousarray(xg[:, :, 3:13, 3:13]).reshape(B, E, NP)
        q = patch.reshape(B // BPC, BPC, KCP, 2, 128, NP).astype(FP8)
        return q.transpose(2, 3, 4, 0, 1, 5)  # [kcp, t, p, core, s, q]

    xpr = patch_pack(x_r)
    xpi = patch_pack(x_i)

    # f_z weight: [E2, 1536] -> x32 -> K-major fp8, plus a 385th column
    # of row-sums/E2 so the matmul emits the LayerNorm mean directly
    fzw_f = np.asarray(fz_w, np.float32).T * SC          # [1536, E2]
    fzw_a = np.concatenate(
        [fzw_f, fzw_f.mean(axis=1, keepdims=True)], axis=1)  # [1536, E2+1]
    fzw_t = fzw_a.reshape(KCPZ, 2, 128, E2 + 1)
    fzw_pack = np.ascontiguousarray(fzw_t.transpose(0, 2, 1, 3)).astype(FP8)

    # conv weights: BN scale folded, x32, K-major fp8 [KCP, 128, 2, 9, E2]
    def fold(w, b, g, beta, m, v):
        w = np.asarray(w, np.float32)
        scale = np.asarray(g, np.float32) / np.sqrt(
            np.asarray(v, np.float32) + EPS)
        shift = (np.asarray(b, np.float32) - np.asarray(m, np.float32)) \
            * scale + np.asarray(beta, np.float32)
        # [co, ci, 3, 3] -> [ci, tap(dy*3+dx), co]
        wt = (w * scale[:, None, None, None] * SC).transpose(1, 2, 3, 0)
        # [ci, tap, co] -> [kcp, mc, p, ksub, tap, 128]
        wt = wt.reshape(E, 9, E2).reshape(KCP, 2, 128, 9, MC, 128)
        wt = np.ascontiguousarray(wt.transpose(0, 4, 2, 1, 3, 5)).astype(FP8)
        return (wt.reshape(KCP, MC, 128, 2 * 9 * 128),
                shift.reshape(MC, 128).astype(np.float32))
    wr_pack, bshr = fold(wr, br, bnr_g, bnr_b, bnr_m, bnr_v)
    wi_pack, bshi = fold(wi, bi, bni_g, bni_b, bni_m, bni_v)

    fzb_v = np.asarray(fz_b, np.float32) * SC
    fzb = np.concatenate([fzb_v, fzb_v.mean(keepdims=True)]).reshape(
        1, E2 + 1)
    lng = np.asarray(ln_g, np.float32).reshape(1, E2)
    lnb = np.asarray(ln_b, np.float32).reshape(1, E2)
    flags = (bool(np.any(fzb_v)), not bool(np.all(lng == 1.0)),
             bool(np.any(lnb)))

    shared = {
        "fzw": fzw_pack, "wr": wr_pack, "wi": wi_pack,
        "bshr": bshr, "bshi": bshi,
        "ones": np.ones((128, 1), BF16),
        "c": np.asarray(c, np.float32).reshape(1, 1),
        "fzb": fzb, "lng": lng, "lnb": lnb,
    }

    zq = z.astype(FP8)
    in_maps = []
    for core in range(N_CORES):
        sl = slice(core * BPC, (core + 1) * BPC)
        m = dict(shared)
        zc = zq[sl].reshape(TOK, TWOE).T.reshape(KCPZ, 2, 128, 2, TOK // 2)
        m["z"] = np.ascontiguousarray(zc.transpose(3, 0, 2, 1, 4))
        m["xr"] = np.ascontiguousarray(xpr[:, :, :, core]).reshape(
            KCP, 2, 128, BPC * NP)
        m["xi"] = np.ascontiguousarray(xpi[:, :, :, core]).reshape(
            KCP, 2, 128, BPC * NP)
        in_maps.append(m)
    return flags, in_maps


def kernel(**inputs):
    from concourse.bass_utils import run_bass_kernel_spmd

    flags, in_maps = prep_inputs(**inputs)
    nc = get_program(flags)
    res = run_bass_kernel_spmd(nc, in_maps, core_ids=list(range(N_CORES)))
    s1 = np.concatenate([np.asarray(res.results[i]["s1"]).reshape(-1)
                         for i in range(N_CORES)])
    s2 = np.concatenate([np.asarray(res.results[i]["s2"]).reshape(-1)
                         for i in range(N_CORES)])
    return (s1.reshape(B, 1, 1, 1).astype(np.float32),
            s2.reshape(B, 1, 1, 1).astype(np.float32))


# revision 75
# speedup vs baseline: 1.0590x; 1.0590x over previous
"""Trainium2 Bass kernel for nn_Cross_classifier (dense_cnn).

Pure data-parallel: batch 128 sharded across 8 NeuronCores (16 samples/core).
All parameters replicated. Self-contained: shapes hardcoded.

Math (mirrors the reference):
  - f_z: Linear(1536->384) + LayerNorm + GELU on z = concat(z_r, z_i).
  - down_r/down_i: 3x3 SAME conv (768->384) + eval-BN + GELU, center-crop
    16x16 -> 8x8.  Only the central 8x8 outputs are consumed, so the conv is
    computed only there from the central 10x10 input patch.  BN scale folds
    into the conv weights; conv bias + BN shift fold into one per-channel
    bias applied inside the GELU activation.
  - xcorr: VALID correlation of an 8x8 kernel over an 8x8 map = per-sample
    dot over (384 ch x 64 pos); then sigmoid(dot / c).

Implementation:
  - Every matmul runs fp8e4m3 x fp8e4m3 in MatmulPerfMode.DoubleRow (two
    128-deep K-subtiles per pass), accumulating in fp32 PSUM.  The final
    sigmoid sits at ~sigmoid(10), so fp8 rounding is far inside tolerance.
    Weights are scaled x32 on the host to center them in fp8e4m3's normal
    range; the scale is removed exactly (LayerNorm is scale-invariant for
    f_z; the conv GELU applies scale=1/32).
  - All layout work (transposes to contraction-major, weight folding, fp8
    casts) happens on the host, so the device program is load -> matmul ->
    activation -> reduce, plus one small bf16 xbar transpose of z_f.
  - Conv moving APs must fit TENSOR3D (3 free dims): per-sample tap windows
    [K, ksub(2), row(8), col(8)] over a 10x10 patch, 64-wide output slices
    of a shared PSUM tile per sample.
  - Engine program order keeps the serial DMA stream and the PE stream in
    lockstep: warmup matmuls burn the PE p-state ramp while the first
    chunk loads, then conv-r k0/k1, f_z, conv-r k2, conv-i.  The LayerNorm
    mean comes free from a 385th fzw column of row-sums; variance uses one
    fused square-accumulate per token tile; the normalize folds into the
    GELU's per-partition bias/scale.  The final sigmoids run as
    0.5*(1+tanh(x/2)) so they share the GELU activation table (the Sqrt
    table is preloaded at t=0), keeping table loads off the critical path.
"""

import numpy as np
import ml_dtypes

N_CORES = 8
B = 128
BPC = B // N_CORES      # samples per core: 16
T1 = 64                 # template tokens (8x8)
E = 768
E2 = 384
TWOE = 2 * E            # 1536
KCPZ = TWOE // 256      # 6 DoubleRow K-chunks for f_z
KCP = E // 256          # 3 DoubleRow K-chunks for conv
MC = E2 // 128          # 3 output-channel chunks
TOK = BPC * T1          # 1024 z tokens per core
NZT = TOK // 128        # 8 token tiles
GRP = BPC // 8          # sample groups of 8 (one PSUM tile each)
NP = 100                # patch elems/sample: 10 rows x 10 cols
SC = 32.0               # fp8 weight scale
EPS = 1e-5

BF16 = ml_dtypes.bfloat16
FP8 = ml_dtypes.float8_e4m3

_PROG_CACHE: dict = {}


def _build_program(flags):
    """flags = (has_fzb, has_lng, has_lnb): whether the f_z linear bias /
    LayerNorm gain / LayerNorm bias are non-trivial (structurally zero/one
    in this model; general path kept for robustness)."""
    from contextlib import ExitStack
    import concourse.bass as bass
    import concourse.mybir as mybir
    import concourse.tile as tile
    from concourse import bacc

    has_fzb, has_lng, has_lnb = flags
    dt = mybir.dt
    f32, bf16, fp8 = dt.float32, dt.bfloat16, dt.float8e4
    AX = mybir.AxisListType
    OP = mybir.AluOpType
    AF = mybir.ActivationFunctionType
    DR = mybir.MatmulPerfMode.DoubleRow

    nc = bacc.Bacc("TRN2", target_bir_lowering=False, debug=False,
                   num_devices=N_CORES)

    # ---- DRAM I/O (layouts: every DMA <=3 affine dims, >=512B runs) ----
    z_d = nc.dram_tensor("z", [2, KCPZ, 128, 2, TOK // 2], fp8,
                         kind="ExternalInput")
    fzw_d = nc.dram_tensor("fzw", [KCPZ, 128, 2, E2 + 1], fp8,
                           kind="ExternalInput")
    xr_d = nc.dram_tensor("xr", [KCP, 2, 128, BPC * NP], fp8,
                          kind="ExternalInput")
    xi_d = nc.dram_tensor("xi", [KCP, 2, 128, BPC * NP], fp8,
                          kind="ExternalInput")
    wr_d = nc.dram_tensor("wr", [KCP, MC, 128, 2 * 9 * 128], fp8,
                          kind="ExternalInput")
    wi_d = nc.dram_tensor("wi", [KCP, MC, 128, 2 * 9 * 128], fp8,
                          kind="ExternalInput")
    bshr_d = nc.dram_tensor("bshr", [MC, 128], f32, kind="ExternalInput")
    bshi_d = nc.dram_tensor("bshi", [MC, 128], f32, kind="ExternalInput")
    ones_d = nc.dram_tensor("ones", [128, 1], bf16, kind="ExternalInput")
    c_d = nc.dram_tensor("c", [1, 1], f32, kind="ExternalInput")
    fzb_d = nc.dram_tensor("fzb", [1, E2 + 1], f32, kind="ExternalInput")
    lng_d = nc.dram_tensor("lng", [1, E2], f32, kind="ExternalInput")
    lnb_d = nc.dram_tensor("lnb", [1, E2], f32, kind="ExternalInput")
    s1_d = nc.dram_tensor("s1", [1, BPC], f32, kind="ExternalOutput")
    s2_d = nc.dram_tensor("s2", [1, BPC], f32, kind="ExternalOutput")

    def bcast_ap(handle):
        ap = handle.ap()
        return bass.AP(tensor=ap.tensor, offset=ap.offset,
                       ap=[[0, 128]] + [list(d) for d in ap.ap[1:]])

    with tile.TileContext(nc, pool_alloc_mode="queue") as tc, ExitStack() as ctx:
        const = ctx.enter_context(tc.tile_pool(name="const", bufs=1))

        # consts ride the DVE ring so the SP ring starts the big loads at
        # t=0; each is tiny and slots between big transfers.
        onesb = const.tile([128, 1], bf16)
        nc.scalar.dma_start(out=onesb, in_=ones_d.ap())
        ctile = const.tile([1, 1], f32)
        nc.scalar.dma_start(out=ctile, in_=c_d.ap())
        invc2 = const.tile([1, 1], f32)
        nc.vector.tensor_scalar_mul(out=invc2, in0=ctile, scalar1=2.0)
        nc.vector.reciprocal(invc2, invc2)
        bshr = const.tile([128, MC], f32)
        nc.scalar.dma_start(out=bshr, in_=bshr_d.ap().rearrange("m p -> p m"))
        bshi = const.tile([128, MC], f32)
        nc.scalar.dma_start(out=bshi, in_=bshi_d.ap().rearrange("m p -> p m"))
        epst = const.tile([128, 1], f32)
        nc.vector.memset(epst, EPS * SC * SC)  # eps for x32-scaled variance
        sqwarm = const.tile([128, 1], f32)
        nc.scalar.activation(out=sqwarm, in_=epst, func=AF.Sqrt)

        if has_fzb:
            fzb_bc = const.tile([128, E2 + 1], f32)
            nc.scalar.dma_start(out=fzb_bc, in_=bcast_ap(fzb_d))
        if has_lng:
            lng_bc = const.tile([128, E2], f32)
            nc.scalar.dma_start(out=lng_bc, in_=bcast_ap(lng_d))
        if has_lnb:
            lnb_bc = const.tile([128, E2], f32)
            nc.scalar.dma_start(out=lnb_bc, in_=bcast_ap(lnb_d))

        # ---- persistent SBUF tiles ----
        data = ctx.enter_context(tc.tile_pool(name="data", bufs=1))
        zt = data.tile([128, 2, KCPZ, 2, TOK // 2], fp8)  # z.T, K-major
        fzw = data.tile([128, KCPZ, 2, E2 + 1], fp8)
        XTr = data.tile([128, KCP, 2, BPC * NP], fp8)
        XTi = data.tile([128, KCP, 2, BPC * NP], fp8)
        Wr = data.tile([128, KCP, MC, 2, 9, 128], fp8)
        Wi = data.tile([128, KCP, MC, 2, 9, 128], fp8)
        zg2all = data.tile([128, NZT, E2], bf16)        # gelu(LN(f_z))
        ZGT = data.tile([128, NZT, MC, 128], bf16)      # ch-major z_f
        zlin = data.tile([128, NZT, E2 + 1], f32)       # f_z lin out + mu

        fin_pool = ctx.enter_context(tc.tile_pool(name="fin", bufs=1))
        cps = ctx.enter_context(tc.tile_pool(name="cps", bufs=6, space="PSUM"))
        xgp = ctx.enter_context(tc.tile_pool(name="xg", bufs=12))
        xcp = ctx.enter_context(tc.tile_pool(name="xc", bufs=4))

        # ---- input DMA stream (SP ring, consumption order: conv-r k0,
        # z-phase tensors, conv-r k1/k2, conv-i) ----
        for k in range(2):
            nc.sync.dma_start(
                out=XTr[:, k, :, :],
                in_=xr_d.ap()[k].rearrange("t p q -> p t q"))
            for mc in range(MC):
                nc.sync.dma_start(out=Wr[:, k, mc, :, :, :],
                                  in_=wr_d.ap()[k, mc])
        nc.sync.dma_start(out=fzw,
                          in_=fzw_d.ap().rearrange("k p t e -> p k t e"))
        for h in range(2):
            nc.sync.dma_start(
                out=zt[:, h], in_=z_d.ap()[h].rearrange("k p t n -> p k t n"))
        nc.sync.dma_start(
            out=XTr[:, 2, :, :],
            in_=xr_d.ap()[2].rearrange("t p q -> p t q"))
        for mc in range(MC):
            nc.sync.dma_start(out=Wr[:, 2, mc, :, :, :],
                              in_=wr_d.ap()[2, mc])
        iloads = []
        for k in range(KCP):
            li = nc.sync.dma_start(
                out=XTi[:, k, :, :],
                in_=xi_d.ap()[k].rearrange("t p q -> p t q"))
            if k == 2:
                iloads.append(li)
            for mc in range(MC):
                li = nc.sync.dma_start(out=Wi[:, k, mc, :, :, :],
                                       in_=wi_d.ap()[k, mc])
                if k == 2:
                    iloads.append(li)

        # ---- conv matmul phase (per branch): per-sample DoubleRow taps ----
        def conv_mm(tag, XT, W, pcs, kcps):
            for kcp in kcps:
                for g in range(GRP):
                    for mc in range(MC):
                        if kcp == 0 and (g, mc) not in pcs:
                            pcs[(g, mc)] = cps.tile([128, 512], f32,
                                                    name="pc", tag="pc")
                        pc = pcs[(g, mc)]
                        for si in range(8):
                            s = g * 8 + si
                            for tap in range(9):
                                dy, dx = tap // 3, tap % 3
                                off = (XT.offset + kcp * (2 * BPC * NP)
                                       + s * NP + dy * 10 + dx)
                                rhs = bass.AP(
                                    tensor=XT.tensor, offset=off,
                                    ap=[list(XT.ap[0]), [BPC * NP, 2],
                                        [10, 8], [1, 8]])
                                nc.tensor.matmul(
                                    pc[:, si * T1:(si + 1) * T1],
                                    lhsT=W[:, kcp, mc, :, tap, :],
                                    rhs=rhs,
                                    start=(kcp == 0 and tap == 0),
                                    stop=(kcp == KCP - 1 and tap == 8),
                                    perf_mode=DR,
                                    skip_group_check=True)

        # gelu(conv/32 + shift) frees the PSUM banks early
        def conv_gelu(tag, pcs, bsh, gate=None, skip=()):
            xgs = {}
            for g in range(GRP):
                for mc in range(MC):
                    if (g, mc) in skip:
                        continue
                    xg = xgp.tile([128, 512], bf16, name="xg", tag="xg")
                    inst = nc.scalar.activation(out=xg, in_=pcs[(g, mc)],
                                                func=AF.Gelu,
                                                bias=bsh[:, mc:mc + 1],
                                                scale=1.0 / SC)
                    if gate is not None:
                        tile.add_dep_helper(inst.ins, gate, sync=True,
                                            reason="sqrt before gelu table")
                    xgs[(g, mc)] = xg
                    xgs["last"] = inst
            return xgs

        # xcorr: per-sample dot with z_f (mul on DVE; reduce on Pool for
        # branch r where it hides under conv-i, on DVE for branch i), then
        # the cross-partition sum accumulates straight into a [1, 16] PSUM
        # via tiny ones-matmuls (one per (g, mc), mc-accumulated).
        def conv_xcorr(tag, xgs, dot, red_eng, skip=()):
            for g in range(GRP):
                for mc in range(MC):
                    if (g, mc) in skip:
                        continue
                    xg = xgs[(g, mc)]
                    prod = xcp.tile([128, 4, 128], bf16, name="prod",
                                    tag="prod", bufs=6)
                    nc.vector.tensor_mul(
                        prod, xg.rearrange("p (a b) -> p a b", a=4),
                        ZGT[:, 4 * g:4 * g + 4, mc, :])
                    red = xcp.tile([128, 8], bf16, name="red", tag="red",
                                   bufs=12)
                    with nc.allow_low_precision(
                            reason="64-term bf16 sums; sigmoid(~10) output "
                                   "tolerates ~1e-2 logit error"):
                        red_eng.tensor_reduce(
                            out=red,
                            in_=prod.rearrange("p a b -> p (a b)").rearrange(
                                "p (s q) -> p s q", q=T1),
                            axis=AX.X, op=OP.add)
                    nc.tensor.matmul(dot[:, g * 8:(g + 1) * 8],
                                     lhsT=onesb, rhs=red,
                                     start=(mc == 0), stop=(mc == MC - 1),
                                     skip_group_check=True)

        # PE order: warmup (burn the p-state ramp while the first chunk
        # loads), conv-r k0, f_z (its LN/act/transpose chain then hides
        # under the remaining conv matmuls), conv-r k1/k2, conv-i.
        junk = data.tile([128, 2, 640], fp8)
        nc.vector.memset(junk, 0.0)
        pcs_r: dict = {}
        for mc in range(MC):
            for g in range(GRP):
                pcs_r[(g, mc)] = cps.tile([128, 512], f32, name="pc",
                                          tag="pc")
        for w in range(19):
            nc.tensor.matmul(pcs_r[(0, 0)], lhsT=junk[:, :, 0:128],
                             rhs=junk[:, :, 0:512], start=True, stop=True,
                             perf_mode=DR, skip_group_check=True)
        conv_mm("r", XTr, Wr, pcs_r, [0, 1])

        # ---------------- f_z: Linear + LayerNorm pass 1 ----------------
        zsp = ctx.enter_context(tc.tile_pool(name="zstat", bufs=4))
        zgp = ctx.enter_context(tc.tile_pool(name="zg", bufs=2))
        fzps = ctx.enter_context(
            tc.tile_pool(name="fzps", bufs=2, space="PSUM"))
        # LN stats: mu comes free as fzw's appended 385th column
        # (row-sums / 384); Sum x^2 via one fused square-accumulate per
        # token tile on DVE.
        rst = zsp.tile([128, NZT], f32, tag="rst", bufs=1)
        ssq = zsp.tile([128, NZT], f32, tag="ssq", bufs=1)
        for tt in range(NZT):
            ps = fzps.tile([128, E2 + 1], f32, name="ps", tag="ps")
            for kcp in range(KCPZ):
                nc.tensor.matmul(
                    ps,
                    lhsT=zt[:, tt // 4, kcp, :,
                            (tt % 4) * 128:(tt % 4 + 1) * 128],
                    rhs=fzw[:, kcp, :, :],
                    start=(kcp == 0), stop=(kcp == KCPZ - 1),
                    perf_mode=DR)
            # copy out (on Act: table-free Copy) so the PSUM bank frees
            # fast (2-bank pool); DVE does only the sum-of-squares
            nc.scalar.copy(out=zlin[:, tt, :], in_=ps)
            if has_fzb:
                nc.vector.tensor_add(zlin[:, tt, :], zlin[:, tt, :],
                                     fzb_bc)
            sq_src = zlin[:, tt, :E2]
            sq = zsp.tile([128, E2], bf16, tag="sqs", bufs=2)
            with nc.allow_low_precision(reason="x^2 scratch, accum is f32"):
                nc.vector.scalar_tensor_tensor(
                    out=sq, in0=sq_src, scalar=1.0,
                    in1=sq_src, op0=OP.mult, op1=OP.mult,
                    accum_out=ssq[:, tt:tt + 1])
        mus = zlin[:, :, E2]                       # [128, NZT], stride E2+1
        musq = zsp.tile([128, NZT], f32, tag="musq", bufs=1)
        nc.vector.tensor_tensor(out=musq, in0=mus, in1=mus, op=OP.mult)
        var8 = zsp.tile([128, NZT], f32, tag="var8", bufs=1)
        nc.vector.scalar_tensor_tensor(out=var8, in0=ssq,
                                       scalar=1.0 / E2, in1=musq,
                                       op0=OP.mult, op1=OP.subtract)
        sq_inst = nc.scalar.activation(out=rst, in_=var8,
                                       func=AF.Sqrt, bias=epst, scale=1.0)
        nc.vector.reciprocal(rst, rst)
        # nmr = -mu * rstd; LN then folds into gelu(rstd*x + nmr)
        # (tokens sit on partitions, so mu/rstd are per-partition)
        nmr = zsp.tile([128, NZT], f32, tag="nmr", bufs=1)
        nc.vector.tensor_tensor(out=nmr, in0=mus, in1=rst, op=OP.mult)
        nc.vector.tensor_scalar_mul(out=nmr, in0=nmr, scalar1=-1.0)

        conv_mm("r", XTr, Wr, pcs_r, [2])
        xgs_r = conv_gelu("r", pcs_r, bshr, gate=sq_inst.ins)

        # ---- f_z pass 2: GELU with folded LN affine ----
        zlast = [sq_inst.ins]
        if has_lng or has_lnb:
            zgtmp = zgp.tile([128, NZT, E2], bf16, tag="zgtmp", bufs=1)
            for tt in range(NZT):
                nc.vector.tensor_scalar(out=zgtmp[:, tt, :],
                                        in0=zlin[:, tt, :E2],
                                        scalar1=zlin[:, tt, E2:E2 + 1],
                                        scalar2=rst[:, tt:tt + 1],
                                        op0=OP.subtract, op1=OP.mult)
                if has_lng:
                    nc.vector.tensor_mul(zgtmp[:, tt, :],
                                         zgtmp[:, tt, :], lng_bc)
                if has_lnb:
                    nc.vector.tensor_add(zgtmp[:, tt, :],
                                         zgtmp[:, tt, :], lnb_bc)
            nc.scalar.activation(
                out=zg2all.rearrange("p a b -> p (a b)"),
                in_=zgtmp.rearrange("p a b -> p (a b)"), func=AF.Gelu)
        else:
            for tt in range(NZT):
                zi = nc.scalar.activation(out=zg2all[:, tt, :],
                                          in_=zlin[:, tt, :E2],
                                          func=AF.Gelu,
                                          bias=nmr[:, tt:tt + 1],
                                          scale=rst[:, tt:tt + 1])
                zlast[0] = zi.ins
                if tt == 3 or tt == NZT - 1:
                    # half-transposes (Act ring): sample-group g0's z_f is
                    # ready for xcorr early.  The conv-i chunk-(h+1) loads
                    # wait for the transpose so it gets a deterministic
                    # DMA-mutex slot instead of starving behind the load
                    # stream (the loads have ample slack).
                    h = tt // 4
                    ti = nc.scalar.dma_start_transpose(
                        ZGT[:, 4 * h:4 * h + 4, :, :],
                        zg2all[:, 4 * h:4 * h + 4, :])

        if has_lng or has_lnb:
            nc.scalar.dma_start_transpose(ZGT[:, :, :, :], zg2all)

        pcs_i: dict = {}
        conv_mm("i", XTi, Wi, pcs_i, [0, 1, 2])
        xgs_i = conv_gelu("i", pcs_i, bshi)

        dot_r = fzps.tile([1, BPC], f32, name="dotr", tag="ps")
        conv_xcorr("r", xgs_r, dot_r, nc.vector)
        sg_r = fin_pool.tile([1, BPC], f32, tag="sgr")
        nc.scalar.activation(out=sg_r, in_=dot_r, func=AF.Tanh,
                             scale=invc2[0:1, 0:1])
        nc.vector.tensor_scalar(out=sg_r, in0=sg_r, scalar1=0.5, scalar2=0.5,
                                op0=OP.mult, op1=OP.add)
        nc.sync.dma_start(out=s1_d.ap(), in_=sg_r)

        dot_i = fzps.tile([1, BPC], f32, name="doti", tag="ps")
        conv_xcorr("i", xgs_i, dot_i, nc.vector)
        sg_i = fin_pool.tile([1, BPC], f32, tag="sgi")
        nc.scalar.activation(out=sg_i, in_=dot_i, func=AF.Tanh,
                             scale=invc2[0:1, 0:1])
        nc.vector.tensor_scalar(out=sg_i, in0=sg_i, scalar1=0.5, scalar2=0.5,
                                op0=OP.mult, op1=OP.add)
        nc.sync.dma_start(out=s2_d.ap(), in_=sg_i)

    nc.finalize()
    return nc


def get_program(flags=(False, False, False)):
    if flags not in _PROG_CACHE:
        _PROG_CACHE[flags] = _build_program(flags)
    return _PROG_CACHE[flags]


def prep_inputs(z_r, z_i, x_r, x_i, fz_w, fz_b, ln_g, ln_b,
                wr, br, bnr_g, bnr_b, bnr_m, bnr_v,
                wi, bi, bni_g, bni_b, bni_m, bni_v, c):
    """Host-side sharding + packing. Returns (flags, in_maps)."""
    z_r = np.asarray(z_r, np.float32)
    z_i = np.asarray(z_i, np.float32)
    x_r = np.asarray(x_r, np.float32)
    x_i = np.asarray(x_i, np.float32)

    # template: z = concat(z_r, z_i) -> [B, 64, 1536]
    z = np.concatenate([z_r, z_i], axis=2)

    # search: central 10x10 patch, K-major fp8:
    # [kcp, ksub, p, core, samp, 100]
    def patch_pack(x):
        xg = x.transpose(0, 2, 1).reshape(B, E, 16, 16)
        patch = np.ascontiguousarray(xg[:, :, 3:13, 3:13]).reshape(B, E, NP)
        q = patch.reshape(B // BPC, BPC, KCP, 2, 128, NP).astype(FP8)
        return q.transpose(2, 3, 4, 0, 1, 5)  # [kcp, t, p, core, s, q]

    xpr = patch_pack(x_r)
    xpi = patch_pack(x_i)

    # f_z weight: [E2, 1536] -> x32 -> K-major fp8, plus a 385th column
    # of row-sums/E2 so the matmul emits the LayerNorm mean directly
    fzw_f = np.asarray(fz_w, np.float32).T * SC          # [1536, E2]
    fzw_a = np.concatenate(
        [fzw_f, fzw_f.mean(axis=1, keepdims=True)], axis=1)  # [1536, E2+1]
    fzw_t = fzw_a.reshape(KCPZ, 2, 128, E2 + 1)
    fzw_pack = np.ascontiguousarray(fzw_t.transpose(0, 2, 1, 3)).astype(FP8)

    # conv weights: BN scale folded, x32, K-major fp8 [KCP, 128, 2, 9, E2]
    def fold(w, b, g, beta, m, v):
        w = np.asarray(w, np.float32)
        scale = np.asarray(g, np.float32) / np.sqrt(
            np.asarray(v, np.float32) + EPS)
        shift = (np.asarray(b, np.float32) - np.asarray(m, np.float32)) \
            * scale + np.asarray(beta, np.float32)
        # [co, ci, 3, 3] -> [ci, tap(dy*3+dx), co]
        wt = (w * scale[:, None, None, None] * SC).transpose(1, 2, 3, 0)
        # [ci, tap, co] -> [kcp, mc, p, ksub, tap, 128]
        wt = wt.reshape(E, 9, E2).reshape(KCP, 2, 128, 9, MC, 128)
        wt = np.ascontiguousarray(wt.transpose(0, 4, 2, 1, 3, 5)).astype(FP8)
        return (wt.reshape(KCP, MC, 128, 2 * 9 * 128),
                shift.reshape(MC, 128).astype(np.float32))
    wr_pack, bshr = fold(wr, br, bnr_g, bnr_b, bnr_m, bnr_v)
    wi_pack, bshi = fold(wi, bi, bni_g, bni_b, bni_m, bni_v)

    fzb_v = np.asarray(fz_b, np.float32) * SC
    fzb = np.concatenate([fzb_v, fzb_v.mean(keepdims=True)]).reshape(
        1, E2 + 1)
    lng = np.asarray(ln_g, np.float32).reshape(1, E2)
    lnb = np.asarray(ln_b, np.float32).reshape(1, E2)
    flags = (bool(np.any(fzb_v)), not bool(np.all(lng == 1.0)),
             bool(np.any(lnb)))

    shared = {
        "fzw": fzw_pack, "wr": wr_pack, "wi": wi_pack,
        "bshr": bshr, "bshi": bshi,
        "ones": np.ones((128, 1), BF16),
        "c": np.asarray(c, np.float32).reshape(1, 1),
        "fzb": fzb, "lng": lng, "lnb": lnb,
    }

    zq = z.astype(FP8)
    in_maps = []
    for core in range(N_CORES):
        sl = slice(core * BPC, (core + 1) * BPC)
        m = dict(shared)
        zc = zq[sl].reshape(TOK, TWOE).T.reshape(KCPZ, 2, 128, 2, TOK // 2)
        m["z"] = np.ascontiguousarray(zc.transpose(3, 0, 2, 1, 4))
        m["xr"] = np.ascontiguousarray(xpr[:, :, :, core]).reshape(
            KCP, 2, 128, BPC * NP)
        m["xi"] = np.ascontiguousarray(xpi[:, :, :, core]).reshape(
            KCP, 2, 128, BPC * NP)
        in_maps.append(m)
    return flags, in_maps


def kernel(**inputs):
    from concourse.bass_utils import run_bass_kernel_spmd

    flags, in_maps = prep_inputs(**inputs)
    nc = get_program(flags)
    res = run_bass_kernel_spmd(nc, in_maps, core_ids=list(range(N_CORES)))
    s1 = np.concatenate([np.asarray(res.results[i]["s1"]).reshape(-1)
                         for i in range(N_CORES)])
    s2 = np.concatenate([np.asarray(res.results[i]["s2"]).reshape(-1)
                         for i in range(N_CORES)])
    return (s1.reshape(B, 1, 1, 1).astype(np.float32),
            s2.reshape(B, 1, 1, 1).astype(np.float32))
